# revision 1
# baseline (speedup 1.0000x reference)
"""Trainium2 Bass kernel for nn_Block_86921548136965 (gnn_message_passing).

Transformer block (LN->MHA->LN->MLP) + global neighbor max-pool + BN/GELU +
3-NN inverse-distance interpolation, data-parallel over batch across 8
NeuronCores with an on-device AllGather for the global gather table.

Self-contained: hardcodes shapes/sharding; only needs concourse (bass) + numpy.
"""
import os
import sys

sys.path.insert(0, "/opt/trn_rl_repo")

DEBUG = bool(os.environ.get("BASSK_DEBUG"))

import numpy as np
import ml_dtypes

import concourse.bass as bass
import concourse.bacc as bacc
import concourse.tile as tile
from concourse import mybir
from concourse import bass_utils
from concourse.masks import make_identity

# problem shapes
B, G, C, H = 64, 512, 384, 6
HD = C // H  # 64
N2, K = 128, 32
HID = 4 * C  # 1536
NCORES = 8
BL = B // NCORES  # 8 batches per core
ROWS = B * G  # 32768 global rows

F32 = mybir.dt.float32
F32R = mybir.dt.float32r
BF16 = mybir.dt.bfloat16
I32 = mybir.dt.int32
AX = mybir.AxisListType
OP = mybir.AluOpType
AF = mybir.ActivationFunctionType

BIG = 1.0e30
EPS_LN = 1e-5
EPS_BN = 1e-5
EPS_W = 1e-8

_CACHE = {}


def _build_program(thr_groups):
    nc = bacc.Bacc("TRN2", target_bir_lowering=False, debug=False,
                   num_devices=NCORES)

    # ---------------- DRAM I/O ----------------
    d_x = nc.dram_tensor("x_in", [BL, G, C], F32, kind="ExternalInput")
    d_wqk = nc.dram_tensor("wqkT", [C, 2 * C], BF16, kind="ExternalInput")
    d_qkb = nc.dram_tensor("qk_bias", [128, 6], F32, kind="ExternalInput")
    d_wv = nc.dram_tensor("wvT", [C, C], BF16, kind="ExternalInput")
    d_vbr = nc.dram_tensor("v_bias_rep", [128, C], F32, kind="ExternalInput")
    d_pjT = nc.dram_tensor("projT", [C, C], BF16, kind="ExternalInput")
    d_pbr = nc.dram_tensor("proj_b_rep", [128, C], F32, kind="ExternalInput")
    d_w1 = nc.dram_tensor("wfc1T", [C, HID], BF16, kind="ExternalInput")
    d_f1b = nc.dram_tensor("fc1_bias", [128, 12], F32, kind="ExternalInput")
    d_w2 = nc.dram_tensor("wfc2T", [HID, C], BF16, kind="ExternalInput")
    d_f2br = nc.dram_tensor("fc2_b_rep", [128, C], F32, kind="ExternalInput")
    d_bns = nc.dram_tensor("bn_scale_rep", [128, C], F32, kind="ExternalInput")
    d_bnh = nc.dram_tensor("bn_shift_rep", [128, C], F32, kind="ExternalInput")
    d_rep6 = nc.dram_tensor("rep6", [6, C], F32, kind="ExternalInput")
    d_nidx = nc.dram_tensor("nidx", [128, BL * K], I32, kind="ExternalInput")
    d_cidx = nc.dram_tensor("cidx", [128, BL], I32, kind="ExternalInput")
    d_l1a = nc.dram_tensor("l1aug", [BL, 5, G], F32, kind="ExternalInput")
    d_l2a = nc.dram_tensor("l2aug", [BL, 5, N2], F32, kind="ExternalInput")
    d_out = nc.dram_tensor("out", [BL, G, C], F32, kind="ExternalOutput")
    if DEBUG:
        d_dxn = nc.dram_tensor("dbg_xn", [BL, 128, 4, C], F32,
                               kind="ExternalOutput")
        d_dqkT = nc.dram_tensor("dbg_qkT", [BL, 128, 6, G], F32,
                                kind="ExternalOutput")
        d_dva = nc.dram_tensor("dbg_va", [BL, 128, 4, 6, 65], F32,
                               kind="ExternalOutput")
        d_dE = nc.dram_tensor("dbg_E", [BL, 128, 4, G], F32,
                              kind="ExternalOutput")
        d_dx1 = nc.dram_tensor("dbg_x1", [BL, 128, 4, C], F32,
                               kind="ExternalOutput")
        d_dx2 = nc.dram_tensor("dbg_x2", [BL, G, C], F32, kind="ExternalOutput")
        d_dvis = nc.dram_tensor("dbg_vis", [BL, 128, C], F32,
                                kind="ExternalOutput")
        d_dw = nc.dram_tensor("dbg_w", [BL, 4, 128, N2], F32,
                              kind="ExternalOutput")
        d_dsum = nc.dram_tensor("dbg_sum", [BL, 6, G], F32,
                                kind="ExternalOutput")
        d_dpool = nc.dram_tensor("dbg_pool", [BL, 128, C], F32,
                                 kind="ExternalOutput")

    from contextlib import ExitStack
    with tile.TileContext(nc) as tc:
        with tc.tile_pool(name="cpool", bufs=1) as cp, \
             tc.tile_pool(name="pchain", bufs=2) as pch, \
             tc.tile_pool(name="dram", bufs=1, space="DRAM") as dp:
            stk = ExitStack()
            wp = stk.enter_context(tc.tile_pool(name="wpool", bufs=1))
            wk = stk.enter_context(tc.tile_pool(name="work", bufs=2))
            psp = stk.enter_context(tc.tile_pool(name="ps", bufs=2, space="PSUM"))

            # ---------------- static loads ----------------
            wqk_s = wp.tile([128, 3, 2 * C], BF16)
            nc.sync.dma_start(wqk_s[:], d_wqk.ap().rearrange(
                "(cc p) f -> p cc f", p=128))
            wv_s = wp.tile([128, 3, C], BF16)
            nc.sync.dma_start(wv_s[:], d_wv.ap().rearrange(
                "(cc p) f -> p cc f", p=128))
            pjT_s = wp.tile([128, 3, C], BF16)
            nc.sync.dma_start(pjT_s[:], d_pjT.ap().rearrange(
                "(cc p) f -> p cc f", p=128))
            w1_s = wp.tile([128, 3, HID], BF16)
            nc.sync.dma_start(w1_s[:], d_w1.ap().rearrange(
                "(cc p) f -> p cc f", p=128))
            w2_s = wp.tile([128, 12, C], BF16)
            nc.sync.dma_start(w2_s[:], d_w2.ap().rearrange(
                "(cc p) f -> p cc f", p=128))
            rep6_s = wp.tile([6, C], F32R)
            nc.sync.dma_start(rep6_s[:], d_rep6.ap().bitcast(F32R))

            qkb_s = wp.tile([128, 6], F32)
            nc.sync.dma_start(qkb_s[:], d_qkb.ap())
            f1b_s = wp.tile([128, 12], F32)
            nc.sync.dma_start(f1b_s[:], d_f1b.ap())
            vbr_s = wp.tile([128, C], F32)
            nc.sync.dma_start(vbr_s[:], d_vbr.ap())
            pbr_s = wp.tile([128, C], F32)
            nc.sync.dma_start(pbr_s[:], d_pbr.ap())
            f2br_s = wp.tile([128, C], F32)
            nc.sync.dma_start(f2br_s[:], d_f2br.ap())
            bns_s = cp.tile([128, C], F32)
            nc.sync.dma_start(bns_s[:], d_bns.ap())
            bnh_s = cp.tile([128, C], F32)
            nc.sync.dma_start(bnh_s[:], d_bnh.ap())
            nidx_s = cp.tile([128, BL * K], I32)
            nc.sync.dma_start(nidx_s[:], d_nidx.ap())
            cidx_s = cp.tile([128, BL], I32)
            nc.sync.dma_start(cidx_s[:], d_cidx.ap())
            l1a_s = cp.tile([5, BL, G], F32)
            nc.sync.dma_start(l1a_s[:], d_l1a.ap().rearrange("b r s -> r b s"))
            l2a_s = cp.tile([5, BL, N2], F32)
            nc.sync.dma_start(l2a_s[:], d_l2a.ap().rearrange("b r s -> r b s"))

            ident = cp.tile([128, 128], F32)
            make_identity(nc, ident[:])
            eps_s = cp.tile([128, 1], F32)
            nc.vector.memset(eps_s[:], EPS_LN)

            # internal DRAM
            sums_d = dp.tile([6, G], F32)
            ag_in = [dp.tile([2 * G, C], BF16, name=f"agin{pp}")
                     for pp in range(4)]
            table = dp.tile([ROWS, C], BF16)
            x2d = dp.tile([BL * G, C], F32)

            # persistent state for phase C
            wts = cp.tile([128, 4 * BL, N2], BF16)   # unnormalized W^T
            rsm32 = cp.tile([128, 4 * BL], F32)      # 1/sum(W)
            vis_s = cp.tile([128, BL, C], BF16)
            cent = cp.tile([128, BL, C], BF16)
            last_pool = {}

            # ---- 3NN weight precompute (centers only; before phase A) ----
            for b3 in range(BL):
                for ch3 in range(4):
                    idx3 = b3 * 4 + ch3
                    psd = psp.tile([128, N2], F32, name=f"d2{idx3}",
                                   tag="ps_b")
                    nc.tensor.matmul(psd[:],
                                     l1a_s[:, b3, ch3 * 128:(ch3 + 1) * 128],
                                     l2a_s[:, b3, :], start=True, stop=True)
                    wiv = wk.tile([128, N2], F32, name=f"wi{idx3}", tag="wiv")
                    nc.vector.reciprocal(wiv[:], psd[:])
                    w8 = wk.tile([128, 8], F32, name=f"w8{idx3}", tag="w8")
                    nc.vector.max(w8[:], wiv[:])
                    w0 = wk.tile([128, N2], F32, name=f"w0{idx3}", tag="w0")
                    ssum = wk.tile([128, 1], F32, name=f"ss{idx3}", tag="ss")
                    nc.vector.scalar_tensor_tensor(
                        out=w0[:], in0=wiv[:], scalar=w8[:, 2:3], in1=wiv[:],
                        op0=OP.is_ge, op1=OP.mult, accum_out=ssum[:])
                    nc.vector.reciprocal(rsm32[:, idx3:idx3 + 1], ssum[:])
                    pst = psp.tile([128, N2], F32, name=f"wt{idx3}",
                                   tag="ps_b")
                    nc.tensor.transpose(pst[:], w0[:], ident[:])
                    nc.scalar.copy(wts[:, idx3, :], pst[:])

            # ---- threshold-grouped single-column gathers with ping-pong
            # max chains; group g valid once AllGather g has landed ----
            def emit_gathers(g, pool_, nbufs=2):
                hi = (g + 1) * (NCORES * 2 * G)
                for (gb, gk) in thr_groups[g]:
                    ga = pool_.tile([128, C], BF16, name=f"ga{gb}_{gk}",
                                    tag="gacc", bufs=nbufs)
                    nc.gpsimd.indirect_dma_start(
                        out=ga[:], out_offset=None,
                        in_=table[0:hi, :].opt(),
                        in_offset=bass.IndirectOffsetOnAxis(
                            ap=nidx_s[:, gb * K + gk:gb * K + gk + 1],
                            axis=0))
                    nt = pch.tile([128, C], BF16, name=f"pc{gb}_{gk}",
                                  tag=f"pch{gb}")
                    prev = last_pool.get(gb)
                    nc.vector.tensor_tensor(
                        out=nt[:],
                        in0=(prev[:] if prev is not None else ga[:]),
                        in1=ga[:], op=OP.max)
                    last_pool[gb] = nt

            # =================== PHASE A: transformer ===================
            HBL = BL // 2
            for b in range(BL):
                xr = wk.tile([128, 4, C], F32, name=f"xr{b}", tag="xr")
                nc.sync.dma_start(xr[:], d_x.ap()[b].rearrange(
                    "(ch p) c -> p ch c", p=128))
                # residual base: x + proj_b
                xb = wk.tile([128, 4, C], F32, name=f"xb{b}", tag="xb", bufs=1)
                pb_b = bass.AP(pbr_s.tensor, pbr_s[:].offset,
                               [pbr_s[:].ap[0], [0, 4], pbr_s[:].ap[1]])
                nc.vector.tensor_tensor(out=xb[:], in0=xr[:], in1=pb_b,
                                        op=OP.add)

                # ---- LN1 -> xn (normalized, no affine; affine folded) ----
                xn = wk.tile([128, 4, C], BF16, name=f"xn{b}", tag="xn", bufs=2)
                for ch in range(4):
                    st6 = wk.tile([128, 6], F32, name=f"st{b}{ch}", tag="st")
                    nc.vector.bn_stats(out=st6[:], in_=xr[:, ch, :])
                    mv = wk.tile([128, 2], F32, name=f"mv{b}{ch}", tag="mv")
                    nc.vector.bn_aggr(out=mv[:], in_=st6[:])
                    sd = wk.tile([128, 1], F32, name=f"sd{b}{ch}", tag="sd")
                    nc.scalar.activation(sd[:], mv[:, 1:2], AF.Sqrt, bias=eps_s[:])
                    rs = wk.tile([128, 1], F32, name=f"rg{b}{ch}", tag="rg")
                    nc.vector.reciprocal(rs[:], sd[:])
                    nc.vector.tensor_scalar(out=xn[:, ch, :], in0=xr[:, ch, :],
                                            scalar1=mv[:, 0:1], scalar2=rs[:],
                                            op0=OP.subtract, op1=OP.mult)
                if DEBUG:
                    nc.sync.dma_start(d_dxn.ap()[b], xn[:])
                # ---- transpose xn -> xnT [c, s] (HWDGE xbar, 3D out) ----
                xnT = wk.tile([128, 3, G], BF16, name=f"xnT{b}", tag="xnT", bufs=2)
                for ch in range(4):
                    eng = nc.sync if ch % 2 == 0 else nc.scalar
                    eng.dma_start_transpose(
                        xnT[:, :, ch * 128:(ch + 1) * 128], xn[:, ch, :])

                # ---- qkT = Weff_qk @ xnT + bias ----
                qkT = wk.tile([128, 6, G], BF16, name=f"qkT{b}", tag="qkT", bufs=2)
                for f in range(6):
                    ps1 = psp.tile([128, G], F32, name=f"qk{b}{f}", tag="ps_a")
                    for cc in range(3):
                        nc.tensor.matmul(ps1[:],
                                         wqk_s[:, cc, f * 128:(f + 1) * 128],
                                         xnT[:, cc, :],
                                         start=(cc == 0), stop=(cc == 2))
                    nc.scalar.activation(qkT[:, f, :], ps1[:], AF.Identity,
                                         bias=qkb_s[:, f:f + 1])

                if DEBUG:
                    pass  # dbg_qkT disabled under bf16
                # ---- v = xn @ WvT + bias, stored as vaug [s, h, 65] ----
                vaug = wk.tile([128, 4, 6, 65], BF16, name=f"va{b}", tag="va",
                               bufs=2)
                nc.vector.memset(vaug[:], 1.0)
                for sch in range(4):
                    ps2 = psp.tile([128, C], F32, name=f"v{b}{sch}", tag="ps_b")
                    for cc in range(3):
                        nc.tensor.matmul(ps2[:],
                                         xnT[:, cc, sch * 128:(sch + 1) * 128],
                                         wv_s[:, cc, :],
                                         start=(cc == 0), stop=(cc == 2))
                    nc.vector.tensor_tensor(
                        out=vaug[:, sch, :, 0:64],
                        in0=ps2[:].rearrange("p (h d) -> p h d", h=6),
                        in1=vbr_s[:].rearrange("p (h d) -> p h d", h=6),
                        op=OP.add)

                if DEBUG:
                    pass  # dbg_va disabled under bf16
                # ---- attention per head ----
                oTr = wk.tile([128, 3, G], F32, name=f"oTr{b}", tag="oTr", bufs=1)
                for h in range(6):
                    po = (h % 2) * 64
                    qT = qkT[po:po + 64, h // 2, :]
                    kT = qkT[po:po + 64, 3 + h // 2, :]
                    Eh = wk.tile([128, 4, G], BF16, name=f"E{b}{h}", tag="E", bufs=2)
                    for kc in range(4):
                        ps3 = psp.tile([128, G], F32, name=f"s{b}{h}{kc}",
                                       tag="ps_a")
                        nc.tensor.matmul(ps3[:],
                                         kT[:, kc * 128:(kc + 1) * 128],
                                         qT, start=True, stop=True)
                        nc.scalar.activation(Eh[:, kc, :], ps3[:], AF.Exp)
                    pass
                    ps4 = psp.tile([65, G], F32, name=f"o{b}{h}", tag="ps_c")
                    for kc in range(4):
                        nc.tensor.matmul(ps4[:], vaug[:, kc, h, :],
                                         Eh[:, kc, :],
                                         start=(kc == 0), stop=(kc == 3))
                    nc.scalar.copy(oTr[po:po + 64, h // 2, :], ps4[0:64, :])
                    sm1 = wk.tile([1, G], F32, name=f"sm{b}{h}", tag="sm1",
                                  bufs=2)
                    nc.scalar.copy(sm1[:], ps4[64:65, :])
                    nc.sync.dma_start(sums_d[h:h + 1, :], sm1[:])

                # ---- normalization matrix R, scale oT ----
                sums6 = wk.tile([6, G], F32, name=f"s6{b}", tag="s6", bufs=2)
                nc.sync.dma_start(sums6[:], sums_d[:, :])
                srec = wk.tile([6, G], F32R, name=f"sr{b}", tag="sr", bufs=2)
                with nc.allow_low_precision("fp32r is fp32-width"):
                    nc.vector.reciprocal(srec[:], sums6[:])
                oTs = wk.tile([128, 3, G], BF16, name=f"oTs{b}", tag="oTs", bufs=2)
                for cc in range(3):
                    ps5 = psp.tile([128, G], F32, name=f"R{b}{cc}", tag="ps_a")
                    nc.tensor.matmul(ps5[:], rep6_s[:, cc * 128:(cc + 1) * 128],
                                     srec[:], start=True, stop=True)
                    nc.vector.tensor_tensor(out=oTs[:, cc, :],
                                            in0=oTr[:, cc, :], in1=ps5[:],
                                            op=OP.mult)

                # ---- proj + residual -> x1 ----
                x1 = wk.tile([128, 4, C], F32, name=f"x1{b}", tag="x1", bufs=2)
                for sch in range(4):
                    ps6 = psp.tile([128, C], F32, name=f"pj{b}{sch}",
                                   tag="ps_b")
                    for cc in range(3):
                        nc.tensor.matmul(ps6[:],
                                         oTs[:, cc, sch * 128:(sch + 1) * 128],
                                         pjT_s[:, cc, :],
                                         start=(cc == 0), stop=(cc == 2))
                    nc.vector.tensor_tensor(out=x1[:, sch, :], in0=ps6[:],
                                            in1=xb[:, sch, :], op=OP.add)

                if DEBUG:
                    nc.sync.dma_start(d_dx1.ap()[b], x1[:])
                # ---- LN2 -> xn2 ----
                xn2 = wk.tile([128, 4, C], BF16, name=f"xn2{b}", tag="xn", bufs=2)
                for ch in range(4):
                    st6b = wk.tile([128, 6], F32, name=f"su{b}{ch}", tag="st")
                    nc.vector.bn_stats(out=st6b[:], in_=x1[:, ch, :])
                    mvb = wk.tile([128, 2], F32, name=f"mw{b}{ch}", tag="mv")
                    nc.vector.bn_aggr(out=mvb[:], in_=st6b[:])
                    sdb = wk.tile([128, 1], F32, name=f"se{b}{ch}", tag="sd")
                    nc.scalar.activation(sdb[:], mvb[:, 1:2], AF.Sqrt,
                                         bias=eps_s[:])
                    rsb = wk.tile([128, 1], F32, name=f"rh{b}{ch}", tag="rg")
                    nc.vector.reciprocal(rsb[:], sdb[:])
                    nc.vector.tensor_scalar(out=xn2[:, ch, :], in0=x1[:, ch, :],
                                            scalar1=mvb[:, 0:1], scalar2=rsb[:],
                                            op0=OP.subtract, op1=OP.mult)
                xn2T = wk.tile([128, 3, G], BF16, name=f"x2T{b}", tag="xnT", bufs=2)
                for ch in range(4):
                    eng = nc.sync if ch % 2 == 0 else nc.scalar
                    eng.dma_start_transpose(
                        xn2T[:, :, ch * 128:(ch + 1) * 128], xn2[:, ch, :])

                # ---- fc1 + gelu -> uT ----
                uT = wk.tile([128, 12, G], BF16, name=f"uT{b}", tag="uT", bufs=1)
                for f in range(12):
                    ps7 = psp.tile([128, G], F32, name=f"f1{b}{f}", tag="ps_a")
                    for cc in range(3):
                        nc.tensor.matmul(ps7[:],
                                         w1_s[:, cc, f * 128:(f + 1) * 128],
                                         xn2T[:, cc, :],
                                         start=(cc == 0), stop=(cc == 2))
                    nc.scalar.activation(uT[:, f, :], ps7[:], AF.Gelu,
                                         bias=f1b_s[:, f:f + 1])

                # ---- fc2 + residual -> x2; dump fp32 + bf16 ----
                for sch in range(4):
                    ps8 = psp.tile([128, C], F32, name=f"f2{b}{sch}",
                                   tag="ps_b")
                    for f in range(12):
                        nc.tensor.matmul(ps8[:],
                                         uT[:, f, sch * 128:(sch + 1) * 128],
                                         w2_s[:, f, :],
                                         start=(f == 0), stop=(f == 11))
                    x2c = wk.tile([128, C], F32, name=f"x2{b}{sch}", tag="x2c", bufs=2)
                    nc.vector.tensor_tensor(out=x2c[:], in0=ps8[:],
                                            in1=x1[:, sch, :], op=OP.add)
                    x2f = wk.tile([128, C], F32, name=f"x2f{b}{sch}",
                                  tag="x2f")
                    nc.vector.tensor_tensor(out=x2f[:], in0=x2c[:],
                                            in1=f2br_s[:], op=OP.add)
                    x2b = wk.tile([128, C], BF16, name=f"x2b{b}{sch}",
                                  tag="x2b")
                    nc.vector.tensor_copy(x2b[:], x2f[:])
                    row0 = b * G + sch * 128
                    nc.sync.dma_start(x2d[row0:row0 + 128, :], x2f[:])
                    agr0 = (b % 2) * G + sch * 128
                    nc.sync.dma_start(
                        ag_in[b // 2][agr0:agr0 + 128, :], x2b[:])
                if b % 2 == 1:
                    pp = b // 2
                    nc.gpsimd.collective_compute(
                        "AllGather", OP.bypass,
                        replica_groups=[list(range(NCORES))],
                        ins=[ag_in[pp][:, :]],
                        outs=[table[pp * NCORES * 2 * G:
                                    (pp + 1) * NCORES * 2 * G, :]])
                    if pp >= 1:
                        emit_gathers(pp - 1, wk)

            # =================== PHASE C: gather/pool/3NN ===================
            stk.close()
            stk2 = ExitStack()
            gp = stk2.enter_context(tc.tile_pool(name="gat", bufs=2))
            psc = stk2.enter_context(
                tc.tile_pool(name="psC", bufs=2, space="PSUM"))

            emit_gathers(3, gp, nbufs=4)
            for b in range(BL):
                nc.gpsimd.indirect_dma_start(
                    out=cent[:, b, :], out_offset=None,
                    in_=table.opt(),
                    in_offset=bass.IndirectOffsetOnAxis(
                        ap=cidx_s[:, b:b + 1], axis=0))
            for b in range(BL):
                # BN (x2 & affine folded) + gelu + 0.3*centers
                pb1 = gp.tile([128, C], F32, name=f"pb1{b}", tag="pb1")
                nc.vector.tensor_tensor(out=pb1[:], in0=last_pool[b][:],
                                        in1=bns_s[:], op=OP.mult)
                pb2 = gp.tile([128, C], F32, name=f"pb2{b}", tag="pb2")
                nc.vector.tensor_tensor(out=pb2[:], in0=pb1[:], in1=bnh_s[:],
                                        op=OP.add)
                gl = gp.tile([128, C], F32, name=f"gl{b}", tag="gl")
                nc.scalar.activation(gl[:], pb2[:], AF.Gelu)
                nc.vector.scalar_tensor_tensor(
                    out=vis_s[:, b, :], in0=cent[:, b, :], scalar=0.3,
                    in1=gl[:], op0=OP.mult, op1=OP.add)
                for ch in range(4):
                    idx = b * 4 + ch
                    psi = psc.tile([128, C], F32, name=f"ip{idx}", tag="ps_i")
                    nc.tensor.matmul(psi[:], wts[:, idx, :], vis_s[:, b, :],
                                     start=True, stop=True)
                    x2r = gp.tile([128, C], F32, name=f"x2r{idx}", tag="x2r")
                    row0 = b * G + ch * 128
                    nc.sync.dma_start(x2r[:], x2d[row0:row0 + 128, :])
                    oc = gp.tile([128, C], F32, name=f"oc{idx}", tag="oc")
                    nc.vector.scalar_tensor_tensor(
                        out=oc[:], in0=psi[:], scalar=rsm32[:, idx:idx + 1],
                        in1=x2r[:], op0=OP.mult, op1=OP.add)
                    nc.sync.dma_start(
                        d_out.ap()[b, ch * 128:(ch + 1) * 128, :], oc[:])
            stk2.close()

    nc.compile()
    return nc


def _prep_inputs(x, level1_center, level2_center, ln1_g, ln1_b, qkv_w, proj_w,
                 proj_b, ln2_g, ln2_b, fc1_w, fc1_b, fc2_w, fc2_b, bn_g, bn_b,
                 bn_mean, bn_var, level1_index, level2_index):
    """Build the per-core in_maps (host-side folding + sharding)."""
    f32 = np.float32
    x = np.ascontiguousarray(np.asarray(x, f32))
    l1c = np.asarray(level1_center, f32)
    l2c = np.asarray(level2_center, f32)
    ln1_g = np.asarray(ln1_g, f32); ln1_b = np.asarray(ln1_b, f32)
    ln2_g = np.asarray(ln2_g, f32); ln2_b = np.asarray(ln2_b, f32)
    qkv_w = np.asarray(qkv_w, f32); proj_w = np.asarray(proj_w, f32)
    proj_b = np.asarray(proj_b, f32)
    fc1_w = np.asarray(fc1_w, f32); fc1_b = np.asarray(fc1_b, f32)
    fc2_w = np.asarray(fc2_w, f32); fc2_b = np.asarray(fc2_b, f32)
    bn_g = np.asarray(bn_g, f32); bn_b = np.asarray(bn_b, f32)
    bn_mean = np.asarray(bn_mean, f32); bn_var = np.asarray(bn_var, f32)
    l1i = np.asarray(level1_index).astype(np.int64).reshape(B, N2, K)
    l2i = np.asarray(level2_index).astype(np.int64).reshape(B, N2)

    # remap global row ids to the 4-way split AllGather table layout:
    # table[p*8192 + c*1024 + (b%2)*512 + g] holds core c, batch b=2p+?, row g
    def _remap(r):
        c = r // (BL * G)
        rem = r % (BL * G)
        b = rem // G
        g = rem % G
        return (b // 2) * (NCORES * 2 * G) + c * (2 * G) + (b % 2) * G + g

    l1i = _remap(l1i)
    l2i = _remap(l2i)
    # sort neighbors ascending per (batch, n2-position): max-pool is
    # order-invariant; sorted columns need only a prefix of the table.
    l1i = np.sort(l1i, axis=2)
    colmax = l1i.reshape(NCORES, BL, N2, K).max(axis=2).max(axis=0)  # [BL,K]
    groups = colmax // (NCORES * 2 * G)
    thr_groups = tuple(
        tuple((int(b_), int(k_)) for b_ in range(BL) for k_ in range(K)
              if groups[b_, k_] == g_)
        for g_ in range(4))

    s = HD ** -0.5
    weff = qkv_w * ln1_g[None, :]
    beff = qkv_w @ ln1_b
    weff[:C] *= s
    beff[:C] *= s
    wqkT = np.ascontiguousarray(weff[:2 * C].T.astype(ml_dtypes.bfloat16))
    qk_bias = np.ascontiguousarray(beff[:2 * C].reshape(6, 128).T)
    wvT = np.ascontiguousarray(weff[2 * C:].T.astype(ml_dtypes.bfloat16))
    v_bias_rep = np.ascontiguousarray(
        np.broadcast_to(beff[2 * C:], (128, C)))
    projT = np.ascontiguousarray(proj_w.T.astype(ml_dtypes.bfloat16))
    proj_b_rep = np.ascontiguousarray(np.broadcast_to(proj_b, (128, C)))
    w1eff = fc1_w * ln2_g[None, :]
    f1bias = fc1_b + fc1_w @ ln2_b
    wfc1T = np.ascontiguousarray(w1eff.T.astype(ml_dtypes.bfloat16))
    fc1_bias = np.ascontiguousarray(f1bias.reshape(12, 128).T)
    wfc2T = np.ascontiguousarray(fc2_w.T.astype(ml_dtypes.bfloat16))
    fc2_b_rep = np.ascontiguousarray(np.broadcast_to(fc2_b, (128, C)))
    gs = bn_g / np.sqrt(bn_var + EPS_BN)
    bn_scale_rep = np.ascontiguousarray(
        np.broadcast_to((2.0 * gs).astype(f32), (128, C)))
    bn_shift_rep = np.ascontiguousarray(
        np.broadcast_to((bn_b - bn_mean * gs).astype(f32), (128, C)))
    rep6 = np.zeros((6, C), f32)
    for h in range(H):
        rep6[h, h * HD:(h + 1) * HD] = 1.0

    # 3NN augmented coordinate blocks
    # d2[p, j] = l1.(-2 l2) + |l2|^2 + |l1|^2
    l1n = (l1c ** 2).sum(-1)                                 # [B, G]
    l2n = (l2c ** 2).sum(-1)                                 # [B, N2]
    l1aug = np.empty((B, 5, G), f32)
    l1aug[:, 0:3] = np.transpose(l1c, (0, 2, 1))
    l1aug[:, 3] = 1.0
    l1aug[:, 4] = l1n
    l2aug = np.empty((B, 5, N2), f32)
    l2aug[:, 0:3] = -2.0 * np.transpose(l2c, (0, 2, 1))
    l2aug[:, 3] = l2n
    l2aug[:, 4] = 1.0

    shared = {
        "wqkT": wqkT, "qk_bias": qk_bias, "wvT": wvT,
        "v_bias_rep": v_bias_rep, "projT": projT,
        "proj_b_rep": proj_b_rep, "wfc1T": wfc1T, "fc1_bias": fc1_bias,
        "wfc2T": wfc2T, "fc2_b_rep": fc2_b_rep,
        "bn_scale_rep": bn_scale_rep, "bn_shift_rep": bn_shift_rep,
        "rep6": rep6,
    }
    in_maps = []
    for c in range(NCORES):
        b0 = c * BL
        # nidx: [128, BL*K], col b*K+k = l1i[b0+b, p, k]
        nid = np.ascontiguousarray(
            np.transpose(l1i[b0:b0 + BL], (1, 0, 2)).reshape(128, BL * K)
            .astype(np.int32))
        cid = np.ascontiguousarray(l2i[b0:b0 + BL].T.astype(np.int32))
        m = dict(shared)
        m["x_in"] = np.ascontiguousarray(x[b0:b0 + BL])
        m["nidx"] = nid
        m["cidx"] = cid
        m["l1aug"] = np.ascontiguousarray(l1aug[b0:b0 + BL])
        m["l2aug"] = np.ascontiguousarray(l2aug[b0:b0 + BL])
        in_maps.append(m)
    return in_maps, thr_groups


def get_program(thr_groups=None):
    if thr_groups is None:
        thr_groups = ((), (), (),
                      tuple((b, k) for b in range(BL) for k in range(K)))
    if thr_groups not in _CACHE:
        _CACHE[thr_groups] = _build_program(thr_groups)
    return _CACHE[thr_groups]


def run(in_maps, thr_groups=None, **kw):
    nc = get_program(thr_groups)
    return bass_utils.run_bass_kernel_spmd(
        nc, in_maps, core_ids=list(range(NCORES)), **kw)


def kernel(**inputs):
    in_maps, thr_groups = _prep_inputs(**inputs)
    res = run(in_maps, thr_groups)
    out = np.concatenate([res.results[c]["out"] for c in range(NCORES)],
                         axis=0)
    return out.astype(np.float32)


if __name__ == "__main__":
    np.random.seed(0)
    get_program()
    print("program built + compiled OK")

